# revision 14
# baseline (speedup 1.0000x reference)
"""Causal self-attention Trainium2 Bass kernel.

Problem: B=4, T=2048, C=1024, NH=16, HD=64, fp32.
Sharding: 2D over 8 cores = 4 batches x 2 head-groups (8 heads each).
Each core computes, for its (batch b, head-group g):
    q/k/v = x[b] @ W{q,k,v}[rows_g].T + b{q,k,v}[rows_g]
    causal attention over its 8 heads
    partial_out = y_local @ Wp[:, cols_g].T        (host adds the two
    group partials per batch plus bp).

Schedule (per core): the four 512-wide query slices are processed as
one software-pipelined loop — projections for slice n (PE-heavy, fp16
weights/x), then causal attention for i-tile n (ACT-heavy exp), then
the output projection for those rows — so the Tile scheduler can fill
each engine's stalls with the neighbouring stage's work.

Layouts:
    kT      : [128, T] per head-pair (head-dim on partitions, 2 heads
              stacked 64+64), persistent; qT is a transient [128, 512]
              per-pair tile for the current i-slice.
    S^T     : two K=64 row-tiled fp16 matmuls (heads concurrent in the
              PE array) into one 2-bank PSUM tile [j=128, cols h0|h1].
    v_ext   : [128, 130] per (j-tile, pair): [v_h0 | 1 | v_h1 | 1]; the
              ones column makes the y-matmul (M=65) also emit the
              softmax denominator as PSUM row 64.
    softmax : no max-subtraction (scores are O(3)); exp on ACT; causal
              masking by a DVE multiply with triangle masks on diagonal
              blocks; P is kept in fp16 for the fast PE weight-load path.
    y^T     : [65, 512] PSUM per head; normalized on eviction using
              reciprocal of row 64 broadcast via gpsimd.

All matmuls run in fp16 (weights/x pre-cast on host; q/k/v/P/y are
small-range, and all accumulation is fp32 in PSUM, so total cost is
~3e-4 rel err) — fp16 gets the fast FWL weight-load path and full rate
at any moving width.
Built with bacc.Bacc + compile() so multi-wait instructions are
legalized (walrus allows one sync-wait per engine instruction); PSUM
slot-recycle deps are pre-absorbed into dummy LDWEIGHTS ops so fused
fp32r LDW+MM structs keep a single wait.
"""

import numpy as np
import ml_dtypes

import concourse.bass as bass
import concourse.mybir as mybir
import concourse.tile as tile
from concourse import bacc
from concourse.tile_rust import add_dep_helper

B, T, C = 4, 2048, 1024
NH, HD = 16, 64
HPG = 8            # heads per group (per core)
NPAIR = HPG // 2   # head pairs per core
CL = HPG * HD      # 512 local channels
F32 = mybir.dt.float32
F32R = mybir.dt.float32r
BF16 = mybir.dt.bfloat16
FP16 = mybir.dt.float16
EXP = mybir.ActivationFunctionType.Exp
SCALE = 1.0 / np.sqrt(HD)
N_CORES = 8
MOFF = (0, 512, 896, 1152)     # packed mask offsets, widths 512/384/256/128


def attention_body(tc, outs, ins, t=T):
    nc = tc.nc
    nit = t // 512            # i-tiles (queries) == x slices
    njb = t // 128            # j-blocks (keys)
    nkt = C // 128            # contraction tiles for projections

    xT = ins["xT"]            # [C, t] bf16
    wqT, wkT, wvT = ins["wqT"], ins["wkT"], ins["wvT"]   # [C, CL] bf16
    wpT = ins["wpT"]          # [CL, C] f32
    bq, bk = ins["bq"], ins["bk"]      # [128, NPAIR] f32
    bvt = ins["bvt"]          # [128, CL] f32 (bv tiled across partitions)
    masks = ins["masks"]      # [128, 1408] packed diagonal masks
    out = outs["out"]         # [t, C] f32

    dum = {}

    def _absorb(deps, first_mms):
        """Absorb multi-lane PSUM slot-recycle deps into dummy LDWEIGHTS
        ops (one per dep) so the group's first matmul keeps at most one
        sync-wait (the fused fp32r LDW+MM struct allows only one; the
        wait-elision pass only credits real engine instructions)."""
        deps = [d for d in deps if d is not None]
        for d in deps:
            ld = nc.tensor.ldweights(weights=dum["t"][0:1, 0:1])
            add_dep_helper(ld.ins, d.ins, reason="absorb slot release")
            for mm in first_mms:
                add_dep_helper(mm.ins, ld.ins, sync=False,
                               reason="order after absorber")

    with tc.tile_pool(name="consts", bufs=1) as consts:
        dum["t"] = consts.tile([1, 2], BF16, tag="dum", name="dum")
        nc.vector.memset(dum["t"], 0)
        nc.tensor.ldweights(weights=dum["t"][0:1, 0:1])  # prime dum dep
        mks = consts.tile([128, 1408], FP16, tag="mks", name="mks")
        nc.sync.dma_start(out=mks, in_=masks)
        bq_t = consts.tile([128, NPAIR], F32, tag="bq", name="bq_t")
        nc.sync.dma_start(out=bq_t, in_=bq)
        bk_t = consts.tile([128, NPAIR], F32, tag="bk", name="bk_t")
        nc.sync.dma_start(out=bk_t, in_=bk)
        bvt_t = consts.tile([128, CL], F32, tag="bvt", name="bvt_t")
        nc.sync.dma_start(out=bvt_t, in_=bvt)

        with tc.tile_pool(name="persist", bufs=1) as pers, \
             tc.tile_pool(name="wts", bufs=1) as wts, \
             tc.tile_pool(name="xin", bufs=3) as xin, \
             tc.tile_pool(name="qy", bufs=2) as qy, \
             tc.tile_pool(name="ptp", bufs=3) as ptp, \
             tc.tile_pool(name="sm", bufs=2) as sm, \
             tc.tile_pool(name="ps1", bufs=1, space="PSUM") as ps1, \
             tc.tile_pool(name="psS", bufs=2, space="PSUM") as psS, \
             tc.tile_pool(name="psY", bufs=2, space="PSUM") as psY, \
             tc.tile_pool(name="psO", bufs=1, space="PSUM") as psO:
            kT = [pers.tile([128, t], FP16, tag=f"kT{p}", name=f"kT{p}")
                  for p in range(NPAIR)]
            vext = [pers.tile([128, njb * 256], FP16, tag=f"vext{p}",
                              name=f"vext{p}") for p in range(NPAIR)]
            for p in range(NPAIR):
                ones_view = vext[p][:, :].rearrange(
                    "q (jt two d) -> q jt two d", jt=njb, two=2)[:, :, :, 64:128]
                nc.vector.memset(ones_view, 1.0)

            wq_t, wk_t, wv_t = [], [], []
            for kk in range(nkt):
                for lst, wsrc, tg in ((wq_t, wqT, "wq"), (wk_t, wkT, "wk"),
                                      (wv_t, wvT, "wv")):
                    w = wts.tile([128, CL], FP16, tag=f"{tg}{kk}",
                                 name=f"{tg}{kk}")
                    nc.sync.dma_start(
                        out=w, in_=wsrc[128 * kk:128 * (kk + 1), :])
                    lst.append(w)
            wp_t = []
            for p in range(NPAIR):
                w = wts.tile([128, C], FP16, tag=f"wp{p}", name=f"wp{p}")
                nc.sync.dma_start(
                    out=w, in_=wpT[128 * p:128 * (p + 1), :])
                wp_t.append(w)

            ps1_hist = []   # (evictor, last mm) per ps1 slot (bufs=2)
            psS_hist = []   # ([readers], last S mm) per psS slot (bufs=2)
            psY_hist = []   # ([norm insts], [last y mms]) per group
            psO_hist = []   # (ot copy, last out mm) per psO slot (bufs=1)

            for n in range(nit):
                # ---- projections for slice n (bf16) ----
                xts = []
                for kk in range(nkt):
                    xt = xin.tile([128, 512], FP16, tag=f"x{kk}",
                                  name=f"x{kk}")
                    nc.sync.dma_start(
                        out=xt,
                        in_=xT[128 * kk:128 * (kk + 1),
                               512 * n:512 * (n + 1)])
                    xts.append(xt)

                def group(body_mms, evict_fn, hist=ps1_hist, dist=1):
                    k = len(hist)
                    prev = hist[k - dist] if k >= dist else None
                    mms = body_mms()
                    if prev is not None:
                        _absorb([prev[0], prev[1]], [mms[0]])
                    ev = evict_fn()
                    hist.append((ev, mms[-1]))

                qTs = []
                for p in range(NPAIR):
                    qp = qy.tile([128, 512], FP16, tag=f"qT{p}",
                                 name=f"qT{p}")
                    qTs.append(qp)
                for wt, bt, dsts in ((wq_t, bq_t, "q"), (wk_t, bk_t, "k")):
                    for p in range(NPAIR):
                        ps = ps1.tile([128, 512], F32, tag="ps1",
                                      name="ps1q")

                        def mk(ps=ps, wt=wt, p=p):
                            return [nc.tensor.matmul(
                                ps,
                                lhsT=wt[kk][:, 128 * p:128 * (p + 1)],
                                rhs=xts[kk],
                                start=(kk == 0), stop=(kk == nkt - 1))
                                for kk in range(nkt)]

                        if dsts == "q":
                            def ev(ps=ps, bt=bt, p=p):
                                return nc.vector.tensor_scalar_add(
                                    out=qTs[p], in0=ps,
                                    scalar1=bt[:, p:p + 1])
                        else:
                            def ev(ps=ps, bt=bt, p=p, n=n):
                                return nc.vector.tensor_scalar_add(
                                    out=kT[p][:, 512 * n:512 * (n + 1)],
                                    in0=ps, scalar1=bt[:, p:p + 1])
                        group(mk, ev)
                for tb in range(4):
                    jt = 4 * n + tb
                    ps = ps1.tile([128, CL], F32, tag="ps1", name="ps1v")

                    def mk(ps=ps, tb=tb):
                        return [nc.tensor.matmul(
                            ps,
                            lhsT=xts[kk][:, 128 * tb:128 * (tb + 1)],
                            rhs=wv_t[kk],
                            start=(kk == 0), stop=(kk == nkt - 1))
                            for kk in range(nkt)]

                    def ev(ps=ps, jt=jt):
                        last = None
                        for p in range(NPAIR):
                            dst = vext[p][:, 256 * jt:256 * (jt + 1)
                                          ].rearrange(
                                "q (two d) -> q two d", two=2)[:, :, 0:64]
                            last = nc.vector.tensor_add(
                                out=dst,
                                in0=ps[:, 128 * p:128 * (p + 1)].rearrange(
                                    "q (two d) -> q two d", two=2),
                                in1=bvt_t[:, 128 * p:128 * (p + 1)
                                          ].rearrange(
                                    "q (two d) -> q two d", two=2))
                        return last

                    group(mk, ev)

                # ---- attention for i-tile n ----
                it = n
                njb_i = 4 * it + 4
                yTs = []
                for p in range(NPAIR):
                    ky = len(psY_hist)
                    prevy = psY_hist[ky - 1] if ky >= 1 else None
                    psy = [psY.tile([128, 512], F32, tag="psY",
                                    name="psy")
                           for _ in range(2)]
                    first_ymms, last_ymms, norms = [], [], []
                    for m in range(njb_i):
                        dm = m - 4 * it
                        off = 128 * dm if dm >= 0 else 0
                        w = 512 - off
                        ks = len(psS_hist)
                        prevs = psS_hist[ks - 2] if ks >= 2 else None
                        pss = psS.tile([128, 1024], F32, tag="psS",
                                       name="pss")
                        smms = []
                        for h in range(2):
                            hb = 64 * h
                            smms.append(nc.tensor.matmul(
                                pss[:, 512 * h + off:512 * (h + 1)],
                                lhsT=kT[p][hb:hb + 64,
                                           128 * m:128 * (m + 1)],
                                rhs=qTs[p][hb:hb + 64, off:512],
                                start=True, stop=True))
                        if prevs is not None:
                            _absorb(list(prevs[0]) + [prevs[1]], [smms[0]])
                        pt = ptp.tile([128, 1024], FP16, tag="pt", name="pt")
                        if dm < 0:
                            ex = nc.scalar.activation(
                                out=pt, in_=pss, func=EXP,
                                scale=float(SCALE))
                            rhs = [pt[:, 0:512], pt[:, 512:1024]]
                            psS_hist.append(([ex], smms[-1]))
                        else:
                            pss3 = pss.rearrange(
                                "q (h w) -> q h w", h=2)[:, :, off:512]
                            pt3 = pt.rearrange(
                                "q (h w) -> q h w", h=2)[:, :, off:512]
                            ex = nc.scalar.activation(
                                out=pt3, in_=pss3, func=EXP,
                                scale=float(SCALE))
                            ptm = ptp.tile([128, 2, 512], FP16, tag="ptm",
                                           name="ptm", bufs=2)
                            mi = nc.vector.tensor_mul(
                                out=ptm[:, :, off:512],
                                in0=pt3,
                                in1=mks[:, MOFF[dm]:MOFF[dm] + w
                                        ].unsqueeze(1).broadcast_to(
                                    [128, 2, w]))
                            rhs = [ptm[:, h, off:512] for h in range(2)]
                            psS_hist.append(([ex, mi], smms[-1]))
                        for h in range(2):
                            ymm = nc.tensor.matmul(
                                psy[h][:, off:512],
                                lhsT=vext[p][:, 256 * m + 128 * h:
                                             256 * m + 128 * (h + 1)],
                                rhs=rhs[h],
                                start=(m == 0), stop=(m == njb_i - 1))
                            if m == 0:
                                first_ymms.append(ymm)
                            if m == njb_i - 1:
                                last_ymms.append(ymm)
                    if prevy is not None:
                        _absorb(list(prevy[0]) + list(prevy[1]), first_ymms)
                    yp = qy.tile([128, 512], FP16, tag=f"yT{p}",
                                 name=f"yT{p}")
                    yTs.append(yp)
                    for h in range(2):
                        dn = sm.tile([64, 512], F32, tag="dn", name="dn",
                                     bufs=3)
                        dc = nc.vector.reciprocal(out=dn,
                                                  in_=psy[h][64:128, :])
                        norms.append(dc)
                        norms.append(nc.vector.tensor_mul(
                            out=yp[64 * h:64 * (h + 1), :],
                            in0=psy[h][0:64, :], in1=dn))
                    psY_hist.append((norms, last_ymms))
                # ---- output projection for this i-tile's rows ----
                for tb in range(4):
                    for oh in range(2):
                        ko = len(psO_hist)
                        prevo = psO_hist[ko - 1] if ko >= 1 else None
                        pso = psO.tile([128, 512], F32, tag="psO",
                                       name="pso")
                        omms = [nc.tensor.matmul(
                            pso,
                            lhsT=yTs[p][:, 128 * tb:128 * (tb + 1)],
                            rhs=wp_t[p][:, 512 * oh:512 * (oh + 1)],
                            start=(p == 0), stop=(p == NPAIR - 1))
                            for p in range(NPAIR)]
                        if prevo is not None:
                            _absorb([prevo[0], prevo[1]], [omms[0]])
                        ot = sm.tile([128, 512], F32, tag="ot", name="ot",
                                     bufs=3)
                        oc = nc.vector.tensor_copy(out=ot, in_=pso)
                        nc.sync.dma_start(
                            out=out[512 * n + 128 * tb:
                                    512 * n + 128 * (tb + 1),
                                    512 * oh:512 * (oh + 1)],
                            in_=ot)
                        psO_hist.append((oc, omms[-1]))


def build_nc(t=T):
    nc = bacc.Bacc("TRN2", target_bir_lowering=False, debug=False)
    ins = {
        "xT": nc.dram_tensor("xT", [C, t], FP16, kind="ExternalInput").ap(),
        "wqT": nc.dram_tensor("wqT", [C, CL], FP16,
                              kind="ExternalInput").ap(),
        "wkT": nc.dram_tensor("wkT", [C, CL], FP16,
                              kind="ExternalInput").ap(),
        "wvT": nc.dram_tensor("wvT", [C, CL], FP16,
                              kind="ExternalInput").ap(),
        "wpT": nc.dram_tensor("wpT", [CL, C], FP16, kind="ExternalInput").ap(),
        "bq": nc.dram_tensor("bq", [128, NPAIR], F32,
                             kind="ExternalInput").ap(),
        "bk": nc.dram_tensor("bk", [128, NPAIR], F32,
                             kind="ExternalInput").ap(),
        "bvt": nc.dram_tensor("bvt", [128, CL], F32,
                              kind="ExternalInput").ap(),
        "masks": nc.dram_tensor("masks", [128, 1408], FP16,
                                kind="ExternalInput").ap(),
    }
    outs = {
        "out": nc.dram_tensor("out", [t, C], F32, kind="ExternalOutput").ap(),
    }
    with tile.TileContext(nc) as tc:
        attention_body(tc, outs, ins, t=t)
    nc.compile()
    return nc


def make_masks():
    """Packed multiplicative causal masks for diagonal blocks dm=0..3
    covering computed region [off:512], off=min(128*dm, 256); widths
    512/384/256/256 at offsets MOFF. mask[jj, c] = 1 iff
    jj <= c + off - 128*dm (c relative to off)."""
    mk = np.zeros((128, 1408), np.float16)
    for dm in range(4):
        off = 128 * dm
        w = 512 - off
        cols = np.arange(w)[None, :] + off - 128 * dm
        mk[:, MOFF[dm]:MOFF[dm] + w] = (
            np.arange(128)[:, None] <= cols).astype(np.float16)
    return mk


def make_core_inputs(x, Wq, bq, Wk, bk, Wv, bv, Wp, b, g):
    """Host-side shard + layout prep for core (batch b, head-group g)."""
    rows = slice(CL * g, CL * (g + 1))
    bf = np.float16
    return {
        "xT": np.ascontiguousarray(x[b].T.astype(bf)),
        "wqT": np.ascontiguousarray(Wq[rows, :].T.astype(bf)),
        "wkT": np.ascontiguousarray(Wk[rows, :].T.astype(bf)),
        "wvT": np.ascontiguousarray(Wv[rows, :].T.astype(bf)),
        "wpT": np.ascontiguousarray(Wp[:, rows].T.astype(bf)),
        "bq": np.ascontiguousarray(bq[rows].reshape(NPAIR, 128).T),
        "bk": np.ascontiguousarray(bk[rows].reshape(NPAIR, 128).T),
        "bvt": np.ascontiguousarray(
            np.tile(bv[rows][None, :], (128, 1)).astype(np.float32)),
        "masks": make_masks(),
    }


_NC_CACHE = {}
LAST_RESULTS = None


def kernel(x, Wq, bq, Wk, bk, Wv, bv, Wp, bp):
    global LAST_RESULTS
    from concourse.bass_utils import run_bass_kernel_spmd

    x = np.asarray(x, np.float32)
    Wq, bq = np.asarray(Wq, np.float32), np.asarray(bq, np.float32)
    Wk, bk = np.asarray(Wk, np.float32), np.asarray(bk, np.float32)
    Wv, bv = np.asarray(Wv, np.float32), np.asarray(bv, np.float32)
    Wp, bp = np.asarray(Wp, np.float32), np.asarray(bp, np.float32)

    if "nc" not in _NC_CACHE:
        _NC_CACHE["nc"] = build_nc()
    nc = _NC_CACHE["nc"]

    in_maps = []
    for core in range(N_CORES):
        b, g = core // 2, core % 2
        in_maps.append(make_core_inputs(x, Wq, bq, Wk, bk, Wv, bv, Wp, b, g))

    res = run_bass_kernel_spmd(nc, in_maps, core_ids=list(range(N_CORES)))
    LAST_RESULTS = res

    out = np.empty((B, T, C), np.float32)
    for b in range(B):
        out[b] = res.results[2 * b]["out"] + res.results[2 * b + 1]["out"] + bp
    return out



# revision 16
# speedup vs baseline: 1.0013x; 1.0013x over previous
"""Causal self-attention Trainium2 Bass kernel.

Problem: B=4, T=2048, C=1024, NH=16, HD=64, fp32.
Sharding: 2D over 8 cores = 4 batches x 2 head-groups (8 heads each).
Each core computes, for its (batch b, head-group g):
    q/k/v = x[b] @ W{q,k,v}[rows_g].T + b{q,k,v}[rows_g]
    causal attention over its 8 heads
    partial_out = y_local @ Wp[:, cols_g].T        (host adds the two
    group partials per batch plus bp).

Schedule (per core): the four 512-wide query slices are processed as
one software-pipelined loop — projections for slice n (PE-heavy, fp16
weights/x), then causal attention for i-tile n (ACT-heavy exp), then
the output projection for those rows — so the Tile scheduler can fill
each engine's stalls with the neighbouring stage's work.

Layouts:
    kT      : [128, T] per head-pair (head-dim on partitions, 2 heads
              stacked 64+64), persistent; qT is a transient [128, 512]
              per-pair tile for the current i-slice.
    S^T     : two K=64 row-tiled fp16 matmuls (heads concurrent in the
              PE array) into one 2-bank PSUM tile [j=128, cols h0|h1].
    v_ext   : [128, 130] per (j-tile, pair): [v_h0 | 1 | v_h1 | 1]; the
              ones column makes the y-matmul (M=65) also emit the
              softmax denominator as PSUM row 64.
    softmax : no max-subtraction (scores are O(3)); exp on ACT; causal
              masking by a DVE multiply with triangle masks on diagonal
              blocks; P is kept in fp16 for the fast PE weight-load path.
    y^T     : [65, 512] PSUM per head; normalized on eviction using
              reciprocal of row 64 broadcast via gpsimd.

All matmuls run in fp16 (weights/x pre-cast on host; q/k/v/P/y are
small-range, and all accumulation is fp32 in PSUM, so total cost is
~3e-4 rel err) — fp16 gets the fast FWL weight-load path and full rate
at any moving width.
Built with bacc.Bacc + compile() so multi-wait instructions are
legalized (walrus allows one sync-wait per engine instruction); PSUM
slot-recycle deps are pre-absorbed into dummy LDWEIGHTS ops so fused
fp32r LDW+MM structs keep a single wait.
"""

import numpy as np
import ml_dtypes

import concourse.bass as bass
import concourse.mybir as mybir
import concourse.tile as tile
from concourse import bacc
from concourse.tile_rust import add_dep_helper

B, T, C = 4, 2048, 1024
NH, HD = 16, 64
HPG = 8            # heads per group (per core)
NPAIR = HPG // 2   # head pairs per core
CL = HPG * HD      # 512 local channels
F32 = mybir.dt.float32
F32R = mybir.dt.float32r
BF16 = mybir.dt.bfloat16
FP16 = mybir.dt.float16
EXP = mybir.ActivationFunctionType.Exp
SCALE = 1.0 / np.sqrt(HD)
N_CORES = 8
MOFF = (0, 512, 896, 1152)     # packed mask offsets, widths 512/384/256/128


def attention_body(tc, outs, ins, t=T):
    nc = tc.nc
    nit = t // 512            # i-tiles (queries) == x slices
    njb = t // 128            # j-blocks (keys)
    nkt = C // 128            # contraction tiles for projections

    xT = ins["xT"]            # [C, t] bf16
    wqT, wkT, wvT = ins["wqT"], ins["wkT"], ins["wvT"]   # [C, CL] bf16
    wpT = ins["wpT"]          # [CL, C] f32
    bq, bk = ins["bq"], ins["bk"]      # [128, NPAIR] f32
    bvt = ins["bvt"]          # [128, CL] f32 (bv tiled across partitions)
    masks = ins["masks"]      # [128, 1408] packed diagonal masks
    out = outs["out"]         # [t, C] f32

    dum = {}

    def _absorb(deps, first_mms):
        """Absorb multi-lane PSUM slot-recycle deps into dummy LDWEIGHTS
        ops (one per dep) so the group's first matmul keeps at most one
        sync-wait (the fused fp32r LDW+MM struct allows only one; the
        wait-elision pass only credits real engine instructions)."""
        deps = [d for d in deps if d is not None]
        for d in deps:
            ld = nc.tensor.ldweights(weights=dum["t"][0:1, 0:1])
            add_dep_helper(ld.ins, d.ins, reason="absorb slot release")
            for mm in first_mms:
                add_dep_helper(mm.ins, ld.ins, sync=False,
                               reason="order after absorber")

    with tc.tile_pool(name="consts", bufs=1) as consts:
        dum["t"] = consts.tile([1, 2], BF16, tag="dum", name="dum")
        nc.vector.memset(dum["t"], 0)
        nc.tensor.ldweights(weights=dum["t"][0:1, 0:1])  # prime dum dep
        mks = consts.tile([128, 1408], FP16, tag="mks", name="mks")
        nc.sync.dma_start(out=mks, in_=masks)
        bq_t = consts.tile([128, NPAIR], F32, tag="bq", name="bq_t")
        nc.sync.dma_start(out=bq_t, in_=bq)
        bk_t = consts.tile([128, NPAIR], F32, tag="bk", name="bk_t")
        nc.sync.dma_start(out=bk_t, in_=bk)
        bvt_t = consts.tile([128, CL], F32, tag="bvt", name="bvt_t")
        nc.sync.dma_start(out=bvt_t, in_=bvt)

        with tc.tile_pool(name="persist", bufs=1) as pers, \
             tc.tile_pool(name="wts", bufs=1) as wts, \
             tc.tile_pool(name="xin", bufs=2) as xin, \
             tc.tile_pool(name="qy", bufs=2) as qy, \
             tc.tile_pool(name="ptp", bufs=3) as ptp, \
             tc.tile_pool(name="sm", bufs=2) as sm, \
             tc.tile_pool(name="ps1", bufs=1, space="PSUM") as ps1, \
             tc.tile_pool(name="psS", bufs=2, space="PSUM") as psS, \
             tc.tile_pool(name="psY", bufs=2, space="PSUM") as psY, \
             tc.tile_pool(name="psO", bufs=1, space="PSUM") as psO:
            kT = [pers.tile([128, t], FP16, tag=f"kT{p}", name=f"kT{p}")
                  for p in range(NPAIR)]
            vext = [pers.tile([128, njb * 256], FP16, tag=f"vext{p}",
                              name=f"vext{p}") for p in range(NPAIR)]
            for p in range(NPAIR):
                ones_view = vext[p][:, :].rearrange(
                    "q (jt two d) -> q jt two d", jt=njb, two=2)[:, :, :, 64:128]
                nc.vector.memset(ones_view, 1.0)

            wq_t, wk_t, wv_t = [], [], []
            for kk in range(nkt):
                for lst, wsrc, tg in ((wq_t, wqT, "wq"), (wk_t, wkT, "wk"),
                                      (wv_t, wvT, "wv")):
                    w = wts.tile([128, CL], FP16, tag=f"{tg}{kk}",
                                 name=f"{tg}{kk}")
                    nc.sync.dma_start(
                        out=w, in_=wsrc[128 * kk:128 * (kk + 1), :])
                    lst.append(w)
            wp_t = []
            for p in range(NPAIR):
                w = wts.tile([128, C], FP16, tag=f"wp{p}", name=f"wp{p}")
                nc.sync.dma_start(
                    out=w, in_=wpT[128 * p:128 * (p + 1), :])
                wp_t.append(w)

            ps1_hist = []   # (evictor, last mm) per ps1 slot (bufs=2)
            psS_hist = []   # ([readers], last S mm) per psS slot (bufs=2)
            psY_hist = []   # ([norm insts], [last y mms]) per group
            psO_hist = []   # (ot copy, last out mm) per psO slot (bufs=1)

            for n in range(nit):
                # ---- projections for slice n (bf16) ----
                xts = []
                for kk in range(nkt):
                    xt = xin.tile([128, 512], FP16, tag=f"x{kk}",
                                  name=f"x{kk}")
                    nc.sync.dma_start(
                        out=xt,
                        in_=xT[128 * kk:128 * (kk + 1),
                               512 * n:512 * (n + 1)])
                    xts.append(xt)

                def group(body_mms, evict_fn, hist=ps1_hist, dist=1):
                    k = len(hist)
                    prev = hist[k - dist] if k >= dist else None
                    mms = body_mms()
                    if prev is not None:
                        _absorb([prev[0], prev[1]], [mms[0]])
                    ev = evict_fn()
                    hist.append((ev, mms[-1]))

                qTs = []
                for p in range(NPAIR):
                    qp = qy.tile([128, 512], FP16, tag=f"qT{p}",
                                 name=f"qT{p}")
                    qTs.append(qp)
                for wt, bt, dsts in ((wq_t, bq_t, "q"), (wk_t, bk_t, "k")):
                    for p in range(NPAIR):
                        ps = ps1.tile([128, 512], F32, tag="ps1",
                                      name="ps1q")

                        def mk(ps=ps, wt=wt, p=p):
                            return [nc.tensor.matmul(
                                ps,
                                lhsT=wt[kk][:, 128 * p:128 * (p + 1)],
                                rhs=xts[kk],
                                start=(kk == 0), stop=(kk == nkt - 1))
                                for kk in range(nkt)]

                        if dsts == "q":
                            def ev(ps=ps, bt=bt, p=p):
                                return nc.vector.tensor_scalar_add(
                                    out=qTs[p], in0=ps,
                                    scalar1=bt[:, p:p + 1])
                        else:
                            def ev(ps=ps, bt=bt, p=p, n=n):
                                return nc.vector.tensor_scalar_add(
                                    out=kT[p][:, 512 * n:512 * (n + 1)],
                                    in0=ps, scalar1=bt[:, p:p + 1])
                        group(mk, ev)
                for tb in range(4):
                    jt = 4 * n + tb
                    ps = ps1.tile([128, CL], F32, tag="ps1", name="ps1v")

                    def mk(ps=ps, tb=tb):
                        return [nc.tensor.matmul(
                            ps,
                            lhsT=xts[kk][:, 128 * tb:128 * (tb + 1)],
                            rhs=wv_t[kk],
                            start=(kk == 0), stop=(kk == nkt - 1))
                            for kk in range(nkt)]

                    def ev(ps=ps, jt=jt):
                        last = None
                        for p in range(NPAIR):
                            dst = vext[p][:, 256 * jt:256 * (jt + 1)
                                          ].rearrange(
                                "q (two d) -> q two d", two=2)[:, :, 0:64]
                            last = nc.vector.tensor_add(
                                out=dst,
                                in0=ps[:, 128 * p:128 * (p + 1)].rearrange(
                                    "q (two d) -> q two d", two=2),
                                in1=bvt_t[:, 128 * p:128 * (p + 1)
                                          ].rearrange(
                                    "q (two d) -> q two d", two=2))
                        return last

                    group(mk, ev)

                # ---- attention for i-tile n ----
                it = n
                njb_i = 4 * it + 4
                yTs = []
                for p in range(NPAIR):
                    ky = len(psY_hist)
                    prevy = psY_hist[ky - 1] if ky >= 1 else None
                    psy = [psY.tile([128, 512], F32, tag="psY",
                                    name="psy")
                           for _ in range(2)]
                    first_ymms, last_ymms, norms = [], [], []
                    for m in range(njb_i):
                        dm = m - 4 * it
                        off = 128 * dm if dm >= 0 else 0
                        w = 512 - off
                        ks = len(psS_hist)
                        prevs = psS_hist[ks - 2] if ks >= 2 else None
                        pss = psS.tile([128, 1024], F32, tag="psS",
                                       name="pss")
                        smms = []
                        for h in range(2):
                            hb = 64 * h
                            smms.append(nc.tensor.matmul(
                                pss[:, 512 * h + off:512 * (h + 1)],
                                lhsT=kT[p][hb:hb + 64,
                                           128 * m:128 * (m + 1)],
                                rhs=qTs[p][hb:hb + 64, off:512],
                                start=True, stop=True))
                        if prevs is not None:
                            _absorb(list(prevs[0]) + [prevs[1]], [smms[0]])
                        pt = ptp.tile([128, 1024], FP16, tag="pt", name="pt")
                        if dm < 0:
                            ex = nc.scalar.activation(
                                out=pt, in_=pss, func=EXP,
                                scale=float(SCALE))
                            rhs = [pt[:, 0:512], pt[:, 512:1024]]
                            psS_hist.append(([ex], smms[-1]))
                        else:
                            pss3 = pss.rearrange(
                                "q (h w) -> q h w", h=2)[:, :, off:512]
                            pt3 = pt.rearrange(
                                "q (h w) -> q h w", h=2)[:, :, off:512]
                            ex = nc.scalar.activation(
                                out=pt3, in_=pss3, func=EXP,
                                scale=float(SCALE))
                            ptm = ptp.tile([128, 2, 512], FP16, tag="ptm",
                                           name="ptm", bufs=2)
                            mi = nc.vector.tensor_mul(
                                out=ptm[:, :, off:512],
                                in0=pt3,
                                in1=mks[:, MOFF[dm]:MOFF[dm] + w
                                        ].unsqueeze(1).broadcast_to(
                                    [128, 2, w]))
                            rhs = [ptm[:, h, off:512] for h in range(2)]
                            psS_hist.append(([ex, mi], smms[-1]))
                        for h in range(2):
                            ymm = nc.tensor.matmul(
                                psy[h][:, off:512],
                                lhsT=vext[p][:, 256 * m + 128 * h:
                                             256 * m + 128 * (h + 1)],
                                rhs=rhs[h],
                                start=(m == 0), stop=(m == njb_i - 1))
                            if m == 0:
                                first_ymms.append(ymm)
                            if m == njb_i - 1:
                                last_ymms.append(ymm)
                    if prevy is not None:
                        _absorb(list(prevy[0]) + list(prevy[1]), first_ymms)
                    yp = qy.tile([128, 512], FP16, tag=f"yT{p}",
                                 name=f"yT{p}")
                    yTs.append(yp)
                    for h in range(2):
                        dn = sm.tile([64, 512], F32, tag="dn", name="dn",
                                     bufs=3)
                        dc = nc.vector.reciprocal(out=dn,
                                                  in_=psy[h][64:128, :])
                        norms.append(dc)
                        norms.append(nc.vector.tensor_mul(
                            out=yp[64 * h:64 * (h + 1), :],
                            in0=psy[h][0:64, :], in1=dn))
                    psY_hist.append((norms, last_ymms))
                # ---- output projection for this i-tile's rows ----
                for tb in range(4):
                    for oh in range(2):
                        ko = len(psO_hist)
                        prevo = psO_hist[ko - 1] if ko >= 1 else None
                        pso = psO.tile([128, 512], F32, tag="psO",
                                       name="pso")
                        omms = [nc.tensor.matmul(
                            pso,
                            lhsT=yTs[p][:, 128 * tb:128 * (tb + 1)],
                            rhs=wp_t[p][:, 512 * oh:512 * (oh + 1)],
                            start=(p == 0), stop=(p == NPAIR - 1))
                            for p in range(NPAIR)]
                        if prevo is not None:
                            _absorb([prevo[0], prevo[1]], [omms[0]])
                        ot = sm.tile([128, 512], F32, tag="ot", name="ot",
                                     bufs=3)
                        oc = nc.vector.tensor_copy(out=ot, in_=pso)
                        nc.scalar.dma_start(
                            out=out[512 * n + 128 * tb:
                                    512 * n + 128 * (tb + 1),
                                    512 * oh:512 * (oh + 1)],
                            in_=ot)
                        psO_hist.append((oc, omms[-1]))


def build_nc(t=T):
    nc = bacc.Bacc("TRN2", target_bir_lowering=False, debug=False)
    ins = {
        "xT": nc.dram_tensor("xT", [C, t], FP16, kind="ExternalInput").ap(),
        "wqT": nc.dram_tensor("wqT", [C, CL], FP16,
                              kind="ExternalInput").ap(),
        "wkT": nc.dram_tensor("wkT", [C, CL], FP16,
                              kind="ExternalInput").ap(),
        "wvT": nc.dram_tensor("wvT", [C, CL], FP16,
                              kind="ExternalInput").ap(),
        "wpT": nc.dram_tensor("wpT", [CL, C], FP16, kind="ExternalInput").ap(),
        "bq": nc.dram_tensor("bq", [128, NPAIR], F32,
                             kind="ExternalInput").ap(),
        "bk": nc.dram_tensor("bk", [128, NPAIR], F32,
                             kind="ExternalInput").ap(),
        "bvt": nc.dram_tensor("bvt", [128, CL], F32,
                              kind="ExternalInput").ap(),
        "masks": nc.dram_tensor("masks", [128, 1408], FP16,
                                kind="ExternalInput").ap(),
    }
    outs = {
        "out": nc.dram_tensor("out", [t, C], F32, kind="ExternalOutput").ap(),
    }
    with tile.TileContext(nc) as tc:
        attention_body(tc, outs, ins, t=t)
    nc.compile()
    return nc


def make_masks():
    """Packed multiplicative causal masks for diagonal blocks dm=0..3
    covering computed region [off:512], off=min(128*dm, 256); widths
    512/384/256/256 at offsets MOFF. mask[jj, c] = 1 iff
    jj <= c + off - 128*dm (c relative to off)."""
    mk = np.zeros((128, 1408), np.float16)
    for dm in range(4):
        off = 128 * dm
        w = 512 - off
        cols = np.arange(w)[None, :] + off - 128 * dm
        mk[:, MOFF[dm]:MOFF[dm] + w] = (
            np.arange(128)[:, None] <= cols).astype(np.float16)
    return mk


def make_core_inputs(x, Wq, bq, Wk, bk, Wv, bv, Wp, b, g):
    """Host-side shard + layout prep for core (batch b, head-group g)."""
    rows = slice(CL * g, CL * (g + 1))
    bf = np.float16
    return {
        "xT": np.ascontiguousarray(x[b].T.astype(bf)),
        "wqT": np.ascontiguousarray(Wq[rows, :].T.astype(bf)),
        "wkT": np.ascontiguousarray(Wk[rows, :].T.astype(bf)),
        "wvT": np.ascontiguousarray(Wv[rows, :].T.astype(bf)),
        "wpT": np.ascontiguousarray(Wp[:, rows].T.astype(bf)),
        "bq": np.ascontiguousarray(bq[rows].reshape(NPAIR, 128).T),
        "bk": np.ascontiguousarray(bk[rows].reshape(NPAIR, 128).T),
        "bvt": np.ascontiguousarray(
            np.tile(bv[rows][None, :], (128, 1)).astype(np.float32)),
        "masks": make_masks(),
    }


_NC_CACHE = {}
LAST_RESULTS = None


def kernel(x, Wq, bq, Wk, bk, Wv, bv, Wp, bp):
    global LAST_RESULTS
    from concourse.bass_utils import run_bass_kernel_spmd

    x = np.asarray(x, np.float32)
    Wq, bq = np.asarray(Wq, np.float32), np.asarray(bq, np.float32)
    Wk, bk = np.asarray(Wk, np.float32), np.asarray(bk, np.float32)
    Wv, bv = np.asarray(Wv, np.float32), np.asarray(bv, np.float32)
    Wp, bp = np.asarray(Wp, np.float32), np.asarray(bp, np.float32)

    if "nc" not in _NC_CACHE:
        _NC_CACHE["nc"] = build_nc()
    nc = _NC_CACHE["nc"]

    in_maps = []
    for core in range(N_CORES):
        b, g = core // 2, core % 2
        in_maps.append(make_core_inputs(x, Wq, bq, Wk, bk, Wv, bv, Wp, b, g))

    res = run_bass_kernel_spmd(nc, in_maps, core_ids=list(range(N_CORES)))
    LAST_RESULTS = res

    out = np.empty((B, T, C), np.float32)
    for b in range(B):
        out[b] = res.results[2 * b]["out"] + res.results[2 * b + 1]["out"] + bp
    return out

